# revision 1
# baseline (speedup 1.0000x reference)
"""Trainium2 Bass kernel for nn_LungCancerGRU (GRU H=64, T=15, B=262144 -> logits [B,2]).

Data parallel over 8 NeuronCores (batch sharded, 32768 rows/core).

Per-core layout is "transposed": gate units on SBUF partitions, batch on the
free dimension.  Batch runs in pair-tiles of 1024 rows = two groups (A, B) of
N=512; group A occupies partitions 0..63, group B 64..127 of every [128, 512]
tile, so all engines run at full 128-partition width.

Per timestep t (per pair-tile):
  psum_rz[:, :512] = BD(W_hr^T) @ h + x2_r @ [x_t^A; x_t^B]   (r preact)
  psum_rz[:, 512:] = BD(W_hz^T) @ h + x2_z @ ...              (z preact)
  psum_hgn         = BD(W_hn^T) @ h + b_hh_n (K=1 ones row)   (h-part of n gate)
  psum_n           = x2_n @ [x_t^A; x_t^B]                    (x-part of n gate)
  z   = sigmoid(psum_rz[:, 512:] + bias_z)   ACT, per-partition bias
  r   = sigmoid(psum_rz[:, :512] + bias_r)
  hgn = copy(psum_hgn)                       DVE psum->bf16
  m1  = hgn * r                              DVE bf16 2x
  psum_n += I128 @ m1                        identity-matmul accumulate (PE)
  n   = tanh(psum_n + bias_n)                ACT
  u = h - n; v = z*u; h' = n + v             DVE bf16 2x

BD(W) = blockdiag(W, W) [128,128] serves both groups per matmul stream.  All
biases fold into PE matmuls / ACT per-partition bias vectors.  The rank-1 x
contribution rides a K=2 accumulating matmul from an x-transpose tile
xt [2, 512*15] (layout [group, n*15+t]) loaded by one fully-contiguous DMA.

Hardware constraint honored throughout: each instruction can carry at most
ONE semaphore wait (matmuls: two, split onto LDWEIGHTS).  InstNoOp
"observer" pseudo-instructions (APs used only for dependency wiring, dropped
at lowering) advance each engine's vector clock once per step so that no
real instruction ever needs a second wait.

FC head: logitsT [2,512] per group via PE (stationary W_fc^T), bias added in
the PSUM->SBUF tensor_scalar copy (per-partition bias), DMA'd out through a
transposing access pattern.
"""

import sys

import numpy as np

sys.path.insert(0, "/opt/trn_rl_repo")

B, T, H = 262144, 15, 64
NCORES = 8
BC = B // NCORES          # 32768 rows per core
N = 512                   # batch columns per group
PAIR = 2 * N              # 1024 rows per pair-tile
NPAIR = BC // PAIR        # 32 pair-tiles per core
IL = 2                    # pair-tiles processed in lockstep
XW = T * N                # xt tile free width (7680)

# f32 const tile column map (cf [128, CF_W])
CF_X2 = 0                 # cols 0:384, partitions 0:2 - x2 lhsT per gate
CF_BHHN = 384
CF_BR = 385
CF_BZ = 386
CF_BN = 387
CF_WFC = 388              # cols 388:390
CF_BFC = 390              # col 390, partitions 0:2
CF_ONES = 392             # cols 392:904, partition 0 - ones rhs row
CF_BH2 = 904              # cols 904:1032, partition 0 - [b_hh_n | b_hh_n] lhsT row
CF_W = 1032

_cache = {}


def _build(dt_h_name, reps=1):
    from contextlib import ExitStack

    import concourse.bacc as bacc
    import concourse.mybir as mybir
    from concourse.tile import TileContext

    f32 = mybir.dt.float32
    dt_h = getattr(mybir.dt, dt_h_name)
    Act = mybir.ActivationFunctionType
    Alu = mybir.AluOpType

    nc = bacc.Bacc(None)

    x_in = nc.dram_tensor("x", [BC, T], f32, kind="ExternalInput")
    out_d = nc.dram_tensor("out", [BC, 2], f32, kind="ExternalOutput")
    cbf_in = nc.dram_tensor("cbf", [128, 512], dt_h, kind="ExternalInput")
    cf_in = nc.dram_tensor("cf", [128, CF_W], f32, kind="ExternalInput")

    with TileContext(nc) as tc, ExitStack() as es:
        # ---- constants: one DMA per dtype ----
        cpool = es.enter_context(tc.tile_pool(name="const", bufs=1))
        cbf = cpool.tile([128, 512], dt_h)
        nc.sync.dma_start(cbf[:], cbf_in[:])
        cf = cpool.tile([128, CF_W], f32)
        nc.sync.dma_start(cf[:], cf_in[:])

        bd_g = [cbf[:, 128 * g:128 * (g + 1)] for g in range(3)]
        i128 = cbf[:, 384:512]
        x2_g = [cf[0:2, 128 * g:128 * (g + 1)] for g in range(3)]
        bias_r = cf[:, CF_BR:CF_BR + 1]
        bias_z = cf[:, CF_BZ:CF_BZ + 1]
        bias_n = cf[:, CF_BN:CF_BN + 1]
        wfc = cf[:, CF_WFC:CF_WFC + 2]
        bfc = cf[0:2, CF_BFC:CF_BFC + 1]
        ones_row = cf[0:1, CF_ONES:CF_ONES + N]
        bh2_row = cf[0:1, CF_BH2:CF_BH2 + 128]

        # ---- pools ----
        xt_pool = es.enter_context(tc.tile_pool(name="xt", bufs=3))
        hp = es.enter_context(tc.tile_pool(name="h", bufs=2 * IL))
        hf = es.enter_context(tc.tile_pool(name="hf", bufs=IL))
        rzp = es.enter_context(tc.tile_pool(name="rz", bufs=2 * IL))
        hgp = es.enter_context(tc.tile_pool(name="hg", bufs=2 * IL))
        m1p = es.enter_context(tc.tile_pool(name="m1", bufs=2 * IL))
        np_ = es.enter_context(tc.tile_pool(name="nt", bufs=2 * IL))
        up = es.enter_context(tc.tile_pool(name="u", bufs=2 * IL))
        wp = es.enter_context(tc.tile_pool(name="w", bufs=2 * IL))
        zhp = es.enter_context(tc.tile_pool(name="zh", bufs=2 * IL))
        stp = es.enter_context(tc.tile_pool(name="stage", bufs=2 * IL))
        prz = es.enter_context(tc.tile_pool(name="prz", bufs=2, space="PSUM"))
        pn = es.enter_context(tc.tile_pool(name="pn", bufs=2, space="PSUM"))
        phgn = es.enter_context(tc.tile_pool(name="phgn", bufs=2, space="PSUM"))
        plog = pn  # FC logits rotate through the pn slots (shared tag)

        def mm(out, lhsT, rhs, start, stop):
            nc.tensor.matmul(out, lhsT, rhs, start=start, stop=stop,
                             skip_group_check=True)

        # ---- engine warm-ups: fold the const-DMA sems into each engine's
        # clock once so steady-state instructions never re-wait on them.
        pwarm = plog.tile([2, 2], f32, tag="pn")
        mm(pwarm[:], cf[0:2, 0:2], cf[0:2, 0:2], True, True)
        pwarm2 = plog.tile([2, 2], f32, tag="pn")
        mm(pwarm2[:], cbf[0:2, 0:2], cbf[0:2, 0:2], True, True)
        wt = cpool.tile([2, 8], f32)
        nc.vector.tensor_copy(wt[0:1, 0:1], cf[0:1, 0:1])
        nc.vector.tensor_copy(wt[0:1, 1:2], cbf[0:1, 0:1])
        nc.scalar.copy(wt[0:1, 2:3], cf[0:1, 0:1])
        nc.scalar.copy(wt[0:1, 3:4], cbf[0:1, 0:1])

        def stage_a(pr, t):
            """PE matmuls + sigmoids + psum->sbuf copy of hgn."""
            h = pr["h"]
            xcols = pr["xtv"][:, t, :]
            p_rz = prz.tile([128, 2 * N], f32, tag="prz")
            p_n = pn.tile([128, N], f32, tag="pn")
            p_h = phgn.tile([128, N], f32, tag="phgn")
            if h is not None:
                mm(p_rz[:, 0:N], bd_g[0], h[:], True, False)
                mm(p_rz[:, 0:N], x2_g[0], xcols, False, True)
                mm(p_rz[:, N:2 * N], bd_g[1], h[:], True, False)
                mm(p_rz[:, N:2 * N], x2_g[1], xcols, False, True)
                mm(p_h[:], bd_g[2], h[:], True, False)
                mm(p_h[:], bh2_row, ones_row, False, True)
            else:
                mm(p_rz[:, 0:N], x2_g[0], xcols, True, True)
                mm(p_rz[:, N:2 * N], x2_g[1], xcols, True, True)
                mm(p_h[:], bh2_row, ones_row, True, True)
            mm(p_n[:], x2_g[2], xcols, True, False)

            rz = rzp.tile([128, 2 * N + 8], dt_h, tag="rz")
            if pr["absorb"] is not None:
                # tiny first-toucher: carries this tile's WAR and advances
                # ACT's DVE clock past the newest h tick -> all other ACT ops
                # this step keep a single PE wait.
                nc.scalar.copy(rz[0:1, 2 * N:2 * N + 1], pr["absorb"][0:1, 0:1])
            # r first (the critical path runs through it), then z
            nc.scalar.activation(rz[:, 0:N], p_rz[:, 0:N], Act.Sigmoid, bias=bias_r)
            nc.scalar.activation(rz[:, N:2 * N], p_rz[:, N:2 * N], Act.Sigmoid,
                                 bias=bias_z)
            hgn_sb = hgp.tile([128, N + 8], dt_h, tag="hg")
            nc.vector.tensor_copy(hgn_sb[:, 0:N], p_h[:])
            # advance DVE's ACT clock past sigmoid_r before m1 runs
            nc.vector.tensor_copy(hgn_sb[0:1, N:N + 1], rz[0:1, 0:1])
            pr["p_n"], pr["rz"], pr["hgn"] = p_n, rz, hgn_sb

        def stage_b(pr, t):
            """m1 multiply, identity accumulate, tanh; off-chain w and z*h."""
            p_n, rz, hgn_sb = pr["p_n"], pr["rz"], pr["hgn"]
            m1 = m1p.tile([128, N], dt_h, tag="m1")
            nc.vector.tensor_tensor(m1[:], hgn_sb[:, 0:N], rz[:, 0:N], Alu.mult)
            mm(p_n[:], i128, m1[:], False, True)
            # off the critical path: w = 1 - z, zh = z * h
            w = wp.tile([128, N], dt_h, tag="w")
            nc.vector.tensor_scalar(w[:], rz[:, N:2 * N], -1.0, 1.0,
                                    Alu.mult, Alu.add)
            if pr["h"] is not None:
                zh = zhp.tile([128, N], dt_h, tag="zh")
                nc.vector.tensor_tensor(zh[:], rz[:, N:2 * N], pr["h"][:], Alu.mult)
                pr["zh"] = zh
            else:
                pr["zh"] = None
            pr["w"] = w
            n_t = np_.tile([128, N], dt_h, tag="nt")
            nc.scalar.activation(n_t[:], p_n[:], Act.Tanh, bias=bias_n)
            pr["n_t"] = n_t

        def stage_c(pr, t):
            """h' = n*(1-z) + z*h   (zh precomputed off-chain)."""
            n_t, w, zh = pr["n_t"], pr["w"], pr["zh"]
            last = t == T - 1
            if last:
                h_new = hf.tile([128, N], f32, tag="hf")
            else:
                h_new = hp.tile([128, N], dt_h, tag="h")
            if zh is not None:
                nw = up.tile([128, N], dt_h, tag="nw")
                nc.vector.tensor_tensor(nw[:], n_t[:], w[:], Alu.mult)
                nc.vector.tensor_tensor(h_new[:], nw[:], zh[:], Alu.add)
            else:
                nc.vector.tensor_tensor(h_new[:], n_t[:], w[:], Alu.mult)
            pr["h"] = h_new

        def fc_out(pr, blk):
            h = pr["h"]
            base = pr["base"]
            st = pr["st"]
            for g in range(2):
                p_l = plog.tile([2, N], f32, tag="pn")
                mm(p_l[:], wfc[64 * g:64 * (g + 1), :], h[64 * g:64 * (g + 1), :],
                   True, True)
                stg = st[0:2, g * N:(g + 1) * N]
                nc.vector.tensor_scalar(stg, p_l[:], bfc, None, Alu.add)
            # one DMA for the whole pair: rows base..base+1024 in DRAM match
            # the st column order exactly
            rows = out_d[base:base + PAIR, :]
            nc.sync.dma_start(rows.rearrange("n two -> two n"), st[0:2, :])

        prev_hf = None
        for blk in range(reps * (NPAIR // IL)):
            blk = blk % (NPAIR // IL)
            pairs = []
            for j in range(IL):
                p = blk * IL + j
                base = p * PAIR
                xt = xt_pool.tile([2, XW], f32)
                # one DMA, contiguous innermost on both sides:
                # xt[g, n*T + t] <- x[base + g*N + n, t]
                src = x_in[base:base + PAIR, :]
                nc.sync.dma_start(
                    xt[:].rearrange("g (n t) -> g n t", t=T),
                    src.rearrange("(g n) t -> g n t", g=2))
                st = stp.tile([2, 2 * N], f32, tag="st")
                # tiny first-toucher carries the WAR on the old out-DMA, so
                # the staging writes later keep a single PE wait
                nc.vector.tensor_copy(st[0:1, 0:1], cf[0:1, 0:1])
                pairs.append({"xtv": xt[:].rearrange("g (n t) -> g t n", t=T),
                              "xt": xt, "base": base, "h": None, "st": st,
                              "absorb": None})
            for t in range(T):
                # newest DVE-written tile: h of the last pair (or previous
                # superblock's final h at t=0)
                newest = pairs[-1]["h"] if t > 0 else prev_hf
                pairs[0]["absorb"] = newest
                pairs[1]["absorb"] = None
                for pr in pairs:
                    stage_a(pr, t)
                for pr in pairs:
                    stage_b(pr, t)
                for pr in pairs:
                    stage_c(pr, t)
            for pr in pairs:
                fc_out(pr, blk)
            prev_hf = pairs[-1]["h"]

    nc.compile()
    return nc


def _host_constants(W_ih, W_hh, b_ih, b_hh, W_fc, b_fc, dt_h_np):
    f32 = np.float32
    cbf = np.zeros((128, 512), f32)
    cf = np.zeros((128, CF_W), f32)
    w_in = W_ih[:, 0].astype(f32)
    for g in range(3):
        W = W_hh[64 * g:64 * (g + 1)].astype(f32)          # [64, 64]
        cbf[0:64, 128 * g:128 * g + 64] = W.T
        cbf[64:128, 128 * g + 64:128 * g + 128] = W.T
        wg = w_in[64 * g:64 * (g + 1)]
        cf[0, 128 * g:128 * g + 64] = wg
        cf[1, 128 * g + 64:128 * g + 128] = wg
    cbf[:, 384:512] = np.eye(128, dtype=f32)
    cf[:, CF_BHHN] = np.concatenate([b_hh[128:192]] * 2)
    cf[:, CF_BR] = np.concatenate([(b_ih[0:64] + b_hh[0:64])] * 2)
    cf[:, CF_BZ] = np.concatenate([(b_ih[64:128] + b_hh[64:128])] * 2)
    cf[:, CF_BN] = np.concatenate([b_ih[128:192]] * 2)
    cf[0:64, CF_WFC:CF_WFC + 2] = W_fc.T
    cf[64:128, CF_WFC:CF_WFC + 2] = W_fc.T
    cf[0:2, CF_BFC] = b_fc
    cf[0, CF_ONES:CF_ONES + N] = 1.0
    cf[0, CF_BH2:CF_BH2 + 128] = np.concatenate([b_hh[128:192]] * 2)
    return {"cbf": cbf.astype(dt_h_np), "cf": cf}


def kernel(x, W_ih, W_hh, b_ih, b_hh, W_fc, b_fc, _trace=False, _trace_kwargs=None):
    import ml_dtypes

    from concourse.bass_utils import run_bass_kernel_spmd

    dt_h_name = "bfloat16"
    if dt_h_name not in _cache:
        _cache[dt_h_name] = _build(dt_h_name)
    nc = _cache[dt_h_name]

    consts = _host_constants(W_ih, W_hh, b_ih, b_hh, W_fc, b_fc,
                             ml_dtypes.bfloat16)
    x = np.ascontiguousarray(np.asarray(x, np.float32))
    in_maps = []
    for c in range(NCORES):
        m = {"x": x[c * BC:(c + 1) * BC]}
        m.update(consts)
        in_maps.append(m)
    kw = {}
    if _trace:
        kw["trace"] = True
        if _trace_kwargs:
            kw.update(_trace_kwargs)
    res = run_bass_kernel_spmd(nc, in_maps, list(range(NCORES)), **kw)
    out = np.concatenate([res.results[c]["out"] for c in range(NCORES)], axis=0)
    if _trace:
        return out, res
    return out


if __name__ == "__main__":
    rng = np.random.default_rng(0)
    s = 1.0 / np.sqrt(H)
    inputs = {
        "x": rng.standard_normal((B, T), dtype=np.float32),
        "W_ih": rng.uniform(-s, s, (3 * H, 1)).astype(np.float32),
        "W_hh": rng.uniform(-s, s, (3 * H, H)).astype(np.float32),
        "b_ih": rng.uniform(-s, s, (3 * H,)).astype(np.float32),
        "b_hh": rng.uniform(-s, s, (3 * H,)).astype(np.float32),
        "W_fc": rng.uniform(-s, s, (2, H)).astype(np.float32),
        "b_fc": rng.uniform(-s, s, (2,)).astype(np.float32),
    }
    out = kernel(**inputs)
    print(out.shape, out.dtype, out[:4])



# revision 3
# speedup vs baseline: 2.1252x; 2.1252x over previous
"""Trainium2 Bass kernel for nn_LungCancerGRU (GRU H=64, T=15, B=262144 -> logits [B,2]).

Data parallel over 8 NeuronCores (batch sharded, 32768 rows/core).

Per-core layout: gate units on SBUF partitions, batch on the free dimension.
Batch runs in pair-tiles of 1024 rows = two groups (A, B) of N=512; group A
occupies partitions 0..63, group B 64..127 of every [128, 512] tile.  Two
pair-tiles (IL=2) run in lockstep to hide the recurrence critical path.

All matmuls are bf16 (moving operand dtype determines PE rate; fp32 moving
data costs 4 cycles/column).  x is cast to bf16 once in a wide 128-partition
layout and round-tripped through scratch DRAM so the per-pair transposed
loads are 2 contiguous descriptors instead of 1024 60-byte ones.

Per timestep t (per pair-tile):
  p_rz[:, :512] = BD(W_hr^T) @ h + x2_r @ x_t       (r preact, K=128 + K=2)
  p_rz[:, 512:] = BD(W_hz^T) @ h + x2_z @ x_t       (z preact)
  p_hgn         = BD(W_hn^T) @ h                    (h-part of n gate)
  p_n           = x2_n @ x_t                        (x-part of n gate)
  r   = sigmoid(p_rz[:, :512] + bias_r)             ACT, per-partition bias
  z   = sigmoid(p_rz[:, 512:] + bias_z)
  m1  = (p_hgn + b_hh_n) * r                        DVE scalar_tensor_tensor
  p_n += I128 @ m1                                  identity-matmul accumulate
  n   = tanh(p_n + b_ih_n)                          ACT
  w = 1-z; zh = z*h; nw = n*w; h' = nw + zh         DVE bf16

FC head: logitsT [2, 512] per group via PE (stationary W_fc^T slice), bias
added in the PSUM->SBUF tensor_scalar copy, staged per-superblock and DMA'd
to a transposed [2, BC] bf16 DRAM output; the host transposes back.
"""

import sys

import numpy as np

sys.path.insert(0, "/opt/trn_rl_repo")

B, T, H = 262144, 15, 64
NCORES = 8
BC = B // NCORES          # 32768 rows per core
N = 512                   # batch columns per group
PAIR = 2 * N              # 1024 rows per pair-tile
NPAIR = BC // PAIR        # 32 pair-tiles per core
IL = 2                    # pair-tiles processed in lockstep
XW = T * N                # xt tile free width (7680)
SB = IL * PAIR            # rows per superblock (2048)

_cache = {}


def _build():
    from contextlib import ExitStack

    import concourse.bacc as bacc
    import concourse.mybir as mybir
    from concourse.tile import TileContext

    f32 = mybir.dt.float32
    bf16 = mybir.dt.bfloat16
    Act = mybir.ActivationFunctionType
    Alu = mybir.AluOpType

    nc = bacc.Bacc(None)

    x_in = nc.dram_tensor("x", [BC, T], f32, kind="ExternalInput")
    out_d = nc.dram_tensor("out", [2, BC], bf16, kind="ExternalOutput")
    cbf_in = nc.dram_tensor("cbf", [128, 1024], bf16, kind="ExternalInput")
    cf_in = nc.dram_tensor("cf", [128, 8], f32, kind="ExternalInput")
    xs_d = nc.dram_tensor("xs", [BC, T], bf16, kind="Internal")

    with TileContext(nc) as tc, ExitStack() as es:
        # ---- constants ----
        cpool = es.enter_context(tc.tile_pool(name="const", bufs=1))
        cbf = cpool.tile([128, 1024], bf16)
        nc.sync.dma_start(cbf[:], cbf_in[:])
        cf = cpool.tile([128, 8], f32)
        nc.sync.dma_start(cf[:], cf_in[:])

        bd_g = [cbf[:, 128 * g:128 * (g + 1)] for g in range(3)]
        i128 = cbf[:, 384:512]
        wfc = cbf[:, 512:514]
        x2_g = [cbf[0:2, 514 + 128 * g:514 + 128 * (g + 1)] for g in range(3)]
        bias_r = cf[:, 0:1]
        bias_z = cf[:, 1:2]
        bias_n = cf[:, 2:3]
        b2 = cf[:, 3:4]
        bfc = cf[0:2, 4:5]

        # ---- x pre-pass: f32 wide load -> bf16 cast -> scratch DRAM ----
        xpre = es.enter_context(tc.tile_pool(name="xpre", bufs=1))
        xw = xpre.tile([128, 256 * T], f32)
        nc.sync.dma_start(xw[:], x_in[:].rearrange("(p c) t -> p (c t)", p=128))
        xbw = xpre.tile([128, 256 * T], bf16)
        nc.vector.tensor_copy(xbw[:], xw[:])
        nc.sync.dma_start(xs_d[:].rearrange("(p c) t -> p (c t)", p=128), xbw[:])

        # ---- pools ----
        xt_pool = es.enter_context(tc.tile_pool(name="xt", bufs=3))
        hp = es.enter_context(tc.tile_pool(name="h", bufs=2 * IL))
        rzp = es.enter_context(tc.tile_pool(name="rz", bufs=2 * IL))
        m1p = es.enter_context(tc.tile_pool(name="m1", bufs=2 * IL))
        np_ = es.enter_context(tc.tile_pool(name="nt", bufs=2 * IL))
        wp = es.enter_context(tc.tile_pool(name="w", bufs=2 * IL))
        zhp = es.enter_context(tc.tile_pool(name="zh", bufs=2 * IL))
        nwp = es.enter_context(tc.tile_pool(name="nw", bufs=2 * IL))
        stp = es.enter_context(tc.tile_pool(name="stage", bufs=2))
        prz = es.enter_context(tc.tile_pool(name="prz", bufs=2, space="PSUM"))
        pn = es.enter_context(tc.tile_pool(name="pn", bufs=2, space="PSUM"))
        phgn = es.enter_context(tc.tile_pool(name="phgn", bufs=2, space="PSUM"))
        plog = pn  # FC logits rotate through the pn slots (shared tag)

        def mm(out, lhsT, rhs, start, stop):
            nc.tensor.matmul(out, lhsT, rhs, start=start, stop=stop,
                             skip_group_check=True)

        # ---- engine warm-ups: fold const-DMA sems into each engine's clock
        pwarm = plog.tile([2, 2], f32, tag="pn")
        mm(pwarm[:], cbf[0:2, 0:2], cbf[0:2, 0:2], True, True)
        wt = cpool.tile([2, 8], f32)
        nc.vector.tensor_copy(wt[0:1, 0:1], cf[0:1, 0:1])
        nc.vector.tensor_copy(wt[0:1, 1:2], cbf[0:1, 0:1])
        nc.scalar.copy(wt[0:1, 2:3], cf[0:1, 0:1])
        nc.scalar.copy(wt[0:1, 3:4], cbf[0:1, 0:1])

        def stage_a(pr, t):
            """PE matmuls + sigmoids."""
            h = pr["h"]
            xcols = pr["xtv"][:, t, :]
            p_rz = prz.tile([128, 2 * N], f32, tag="prz")
            p_n = pn.tile([128, N], f32, tag="pn")
            if h is not None:
                p_h = phgn.tile([128, N], f32, tag="phgn")
                mm(p_rz[:, 0:N], bd_g[0], h[:], True, False)
                mm(p_rz[:, 0:N], x2_g[0], xcols, False, True)
                mm(p_rz[:, N:2 * N], bd_g[1], h[:], True, False)
                mm(p_rz[:, N:2 * N], x2_g[1], xcols, False, True)
                mm(p_h[:], bd_g[2], h[:], True, True)
                pr["p_h"] = p_h
            else:
                mm(p_rz[:, 0:N], x2_g[0], xcols, True, True)
                mm(p_rz[:, N:2 * N], x2_g[1], xcols, True, True)
                pr["p_h"] = None
            mm(p_n[:], x2_g[2], xcols, True, False)

            rz = rzp.tile([128, 2 * N], bf16, tag="rz")
            nc.scalar.activation(rz[:, 0:N], p_rz[:, 0:N], Act.Sigmoid, bias=bias_r)
            nc.scalar.activation(rz[:, N:2 * N], p_rz[:, N:2 * N], Act.Sigmoid,
                                 bias=bias_z)
            pr["p_n"], pr["rz"] = p_n, rz

        def stage_b(pr, t):
            """m1 fused mult, identity accumulate, tanh; off-chain w and z*h."""
            p_n, rz, p_h = pr["p_n"], pr["rz"], pr["p_h"]
            m1 = m1p.tile([128, N], bf16, tag="m1")
            if p_h is not None:
                nc.vector.scalar_tensor_tensor(m1[:], p_h[:], b2, rz[:, 0:N],
                                               Alu.add, Alu.mult)
            else:
                nc.vector.tensor_scalar(m1[:], rz[:, 0:N], b2, None, Alu.mult)
            mm(p_n[:], i128, m1[:], False, True)
            w = wp.tile([128, N], bf16, tag="w")
            nc.vector.tensor_scalar(w[:], rz[:, N:2 * N], -1.0, 1.0,
                                    Alu.mult, Alu.add)
            if pr["h"] is not None:
                zh = zhp.tile([128, N], bf16, tag="zh")
                nc.vector.tensor_tensor(zh[:], rz[:, N:2 * N], pr["h"][:], Alu.mult)
                pr["zh"] = zh
            else:
                pr["zh"] = None
            pr["w"] = w
            n_t = np_.tile([128, N], bf16, tag="nt")
            nc.scalar.activation(n_t[:], p_n[:], Act.Tanh, bias=bias_n)
            pr["n_t"] = n_t

        def stage_c(pr, t):
            """h' = n*(1-z) + z*h   (zh precomputed off-chain)."""
            n_t, w, zh = pr["n_t"], pr["w"], pr["zh"]
            h_new = hp.tile([128, N], bf16, tag="h")
            if zh is not None:
                nw = nwp.tile([128, N], bf16, tag="nw")
                nc.vector.tensor_tensor(nw[:], n_t[:], w[:], Alu.mult)
                nc.vector.tensor_tensor(h_new[:], nw[:], zh[:], Alu.add)
            else:
                nc.vector.tensor_tensor(h_new[:], n_t[:], w[:], Alu.mult)
            pr["h"] = h_new

        def fc_out(pr, st, j):
            h = pr["h"]
            for g in range(2):
                p_l = plog.tile([2, N], f32, tag="pn")
                mm(p_l[:], wfc[64 * g:64 * (g + 1), :], h[64 * g:64 * (g + 1), :],
                   True, True)
                stg = st[0:2, j * PAIR + g * N:j * PAIR + (g + 1) * N]
                nc.vector.tensor_scalar(stg, p_l[:], bfc, None, Alu.add)

        for blk in range(NPAIR // IL):
            sbbase = blk * SB
            pairs = []
            st = stp.tile([2, SB], bf16, tag="st")
            for j in range(IL):
                base = sbbase + j * PAIR
                xt = xt_pool.tile([2, XW], bf16)
                # flat contiguous DMA: 2 descriptors of 15KB
                nc.sync.dma_start(
                    xt[:], xs_d[base:base + PAIR, :].rearrange("(g n) t -> g (n t)",
                                                               g=2))
                pairs.append({"xtv": xt[:].rearrange("g (n t) -> g t n", t=T),
                              "base": base, "h": None})
            for t in range(T):
                for pr in pairs:
                    stage_a(pr, t)
                for pr in pairs:
                    stage_b(pr, t)
                for pr in pairs:
                    stage_c(pr, t)
            for j, pr in enumerate(pairs):
                fc_out(pr, st, j)
            nc.sync.dma_start(out_d[0:2, sbbase:sbbase + SB], st[0:2, :])

    nc.compile()
    return nc


def _host_constants(W_ih, W_hh, b_ih, b_hh, W_fc, b_fc):
    import ml_dtypes

    f32 = np.float32
    cbf = np.zeros((128, 1024), f32)
    cf = np.zeros((128, 8), f32)
    w_in = W_ih[:, 0].astype(f32)
    for g in range(3):
        W = W_hh[64 * g:64 * (g + 1)].astype(f32)          # [64, 64]
        cbf[0:64, 128 * g:128 * g + 64] = W.T
        cbf[64:128, 128 * g + 64:128 * g + 128] = W.T
        wg = w_in[64 * g:64 * (g + 1)]
        cbf[0, 514 + 128 * g:514 + 128 * g + 64] = wg
        cbf[1, 514 + 128 * g + 64:514 + 128 * g + 128] = wg
    cbf[:, 384:512] = np.eye(128, dtype=f32)
    cbf[0:64, 512:514] = W_fc.T
    cbf[64:128, 512:514] = W_fc.T
    cf[:, 0] = np.concatenate([(b_ih[0:64] + b_hh[0:64])] * 2)
    cf[:, 1] = np.concatenate([(b_ih[64:128] + b_hh[64:128])] * 2)
    cf[:, 2] = np.concatenate([b_ih[128:192]] * 2)
    cf[:, 3] = np.concatenate([b_hh[128:192]] * 2)
    cf[0:2, 4] = b_fc
    return {"cbf": cbf.astype(ml_dtypes.bfloat16), "cf": cf}


def kernel(x, W_ih, W_hh, b_ih, b_hh, W_fc, b_fc, _trace=False, _trace_kwargs=None):
    from concourse.bass_utils import run_bass_kernel_spmd

    if "nc" not in _cache:
        _cache["nc"] = _build()
    nc = _cache["nc"]

    consts = _host_constants(W_ih, W_hh, b_ih, b_hh, W_fc, b_fc)
    x = np.ascontiguousarray(np.asarray(x, np.float32))
    in_maps = []
    for c in range(NCORES):
        m = {"x": x[c * BC:(c + 1) * BC]}
        m.update(consts)
        in_maps.append(m)
    kw = {}
    if _trace:
        kw["trace"] = True
        if _trace_kwargs:
            kw.update(_trace_kwargs)
    res = run_bass_kernel_spmd(nc, in_maps, list(range(NCORES)), **kw)
    out = np.concatenate(
        [np.asarray(res.results[c]["out"]).astype(np.float32).T
         for c in range(NCORES)], axis=0)
    if _trace:
        return out, res
    return out


if __name__ == "__main__":
    rng = np.random.default_rng(0)
    s = 1.0 / np.sqrt(H)
    inputs = {
        "x": rng.standard_normal((B, T), dtype=np.float32),
        "W_ih": rng.uniform(-s, s, (3 * H, 1)).astype(np.float32),
        "W_hh": rng.uniform(-s, s, (3 * H, H)).astype(np.float32),
        "b_ih": rng.uniform(-s, s, (3 * H,)).astype(np.float32),
        "b_hh": rng.uniform(-s, s, (3 * H,)).astype(np.float32),
        "W_fc": rng.uniform(-s, s, (2, H)).astype(np.float32),
        "b_fc": rng.uniform(-s, s, (2,)).astype(np.float32),
    }
    out = kernel(**inputs)
    print(out.shape, out.dtype, out[:4])


# revision 7
# speedup vs baseline: 2.8195x; 1.3267x over previous
"""Trainium2 Bass kernel for nn_LungCancerGRU (GRU H=64, T=15, B=262144 -> logits [B,2]).

Data parallel over 8 NeuronCores (batch sharded, 32768 rows/core).

Per-core layout: gate units on SBUF partitions, batch on the free dimension.
Batch runs in pair-tiles of 1024 rows = two groups (A, B) of N=512; group A
occupies partitions 0..63, group B 64..127 of every [128, 512] tile.  Two
pair-tiles (IL=2) run in lockstep to hide the recurrence critical path.

All matmuls are bf16 (moving operand dtype determines PE rate; fp32 moving
data costs 4 cycles/column).  x is cast to bf16 once in a wide 128-partition
layout and round-tripped through scratch DRAM so the per-pair transposed
loads are 2 contiguous descriptors instead of 1024 60-byte ones.

Per timestep t (per pair-tile):
  p_rz[:, :512] = BD(W_hr^T) @ h + x2_r @ x_t       (r preact, K=128 + K=2)
  p_rz[:, 512:] = BD(W_hz^T) @ h + x2_z @ x_t       (z preact)
  p_hgn         = BD(W_hn^T) @ h                    (h-part of n gate)
  p_n           = x2_n @ x_t                        (x-part of n gate)
  r   = sigmoid(p_rz[:, :512] + bias_r)             ACT, per-partition bias
  z   = sigmoid(p_rz[:, 512:] + bias_z)
  m1  = (p_hgn + b_hh_n) * r                        DVE scalar_tensor_tensor
  p_n += I128 @ m1                                  identity-matmul accumulate
  n   = tanh(p_n + b_ih_n)                          ACT
  w = 1-z; zh = z*h; nw = n*w; h' = nw + zh         DVE bf16

FC head: logitsT [2, 512] per group via PE (stationary W_fc^T slice), bias
added in the PSUM->SBUF tensor_scalar copy, staged per-superblock and DMA'd
to a transposed [2, BC] bf16 DRAM output; the host transposes back.
"""

import sys

import numpy as np

sys.path.insert(0, "/opt/trn_rl_repo")

B, T, H = 262144, 15, 64
NCORES = 8
BC = B // NCORES          # 32768 rows per core
N = 512                   # batch columns per group
PAIR = 2 * N              # 1024 rows per pair-tile
NPAIR = BC // PAIR        # 32 pair-tiles per core
IL = 2                    # pair-tiles processed in lockstep
XW = T * N                # xt tile free width (7680)
SB = IL * PAIR            # rows per superblock (2048)

_cache = {}


def _build():
    from contextlib import ExitStack

    import concourse.bacc as bacc
    import concourse.mybir as mybir
    from concourse.tile import TileContext

    f32 = mybir.dt.float32
    bf16 = mybir.dt.bfloat16
    Act = mybir.ActivationFunctionType
    Alu = mybir.AluOpType

    nc = bacc.Bacc(None)

    x_in = nc.dram_tensor("x", [BC, T], f32, kind="ExternalInput")
    out_d = nc.dram_tensor("out", [2, BC], bf16, kind="ExternalOutput")
    cbf_in = nc.dram_tensor("cbf", [128, 1024], bf16, kind="ExternalInput")
    cf_in = nc.dram_tensor("cf", [128, 8], f32, kind="ExternalInput")
    # scratch x, bf16, t-major per 512-row group-block: xs[r, t*512+n] = x[512r+n, t]
    xs_d = nc.dram_tensor("xs", [BC // N, XW], bf16, kind="Internal")

    with TileContext(nc) as tc, ExitStack() as es:
        # ---- constants ----
        cpool = es.enter_context(tc.tile_pool(name="const", bufs=1))
        cbf = cpool.tile([128, 1024], bf16)
        nc.sync.dma_start(cbf[:], cbf_in[:])
        cf = cpool.tile([128, 8], f32)
        nc.sync.dma_start(cf[:], cf_in[:])

        bd_g = [cbf[:, 128 * g:128 * (g + 1)] for g in range(3)]
        i128 = cbf[:, 384:512]
        wfc = cbf[:, 512:514]
        x2_g = [cbf[0:2, 514 + 128 * g:514 + 128 * (g + 1)] for g in range(3)]
        bias_r = cf[:, 0:1]
        bias_z = cf[:, 1:2]
        bias_n = cf[:, 2:3]
        b2 = cf[:, 3:4]
        bfc = cf[0:2, 4:5]

        # ---- x pre-pass: f32 wide load -> transposing bf16 cast -> scratch ----
        xpre = es.enter_context(tc.tile_pool(name="xpre", bufs=1))
        xw = xpre.tile([64, XW], f32)
        nc.sync.dma_start(xw[:], x_in[:].rearrange("(p n) t -> p (n t)", p=64))
        xbw = xpre.tile([64, XW], bf16)
        nc.vector.tensor_copy(xbw[:].rearrange("p (t n) -> p t n", n=N),
                              xw[:].rearrange("p (n t) -> p t n", t=T))
        nc.sync.dma_start(xs_d[:], xbw[:])

        # ---- pools ----
        xt_pool = es.enter_context(tc.tile_pool(name="xt", bufs=3))
        hp = es.enter_context(tc.tile_pool(name="h", bufs=2 * IL))
        rzp = es.enter_context(tc.tile_pool(name="rz", bufs=2 * IL))
        m1p = es.enter_context(tc.tile_pool(name="m1", bufs=2 * IL))
        np_ = es.enter_context(tc.tile_pool(name="nt", bufs=2 * IL))
        wp = es.enter_context(tc.tile_pool(name="w", bufs=2 * IL))
        zhp = es.enter_context(tc.tile_pool(name="zh", bufs=2 * IL))
        nwp = es.enter_context(tc.tile_pool(name="nw", bufs=2 * IL))
        stp = es.enter_context(tc.tile_pool(name="stage", bufs=2))
        prz = es.enter_context(tc.tile_pool(name="prz", bufs=2, space="PSUM"))
        pn = es.enter_context(tc.tile_pool(name="pn", bufs=2, space="PSUM"))
        phgn = es.enter_context(tc.tile_pool(name="phgn", bufs=2, space="PSUM"))
        plog = pn  # FC logits rotate through the pn slots (shared tag)

        def mm(out, lhsT, rhs, start, stop):
            nc.tensor.matmul(out, lhsT, rhs, start=start, stop=stop,
                             skip_group_check=True)

        # ---- engine warm-ups: fold const-DMA sems into each engine's clock
        pwarm = plog.tile([2, 2], f32, tag="pn")
        mm(pwarm[:], cbf[0:2, 0:2], cbf[0:2, 0:2], True, True)
        wt = cpool.tile([2, 8], f32)
        nc.vector.tensor_copy(wt[0:1, 0:1], cf[0:1, 0:1])
        nc.vector.tensor_copy(wt[0:1, 1:2], cbf[0:1, 0:1])
        nc.scalar.copy(wt[0:1, 2:3], cf[0:1, 0:1])
        nc.scalar.copy(wt[0:1, 3:4], cbf[0:1, 0:1])

        def stage_a(pr, t):
            """PE matmuls + sigmoids."""
            h = pr["h"]
            xcols = pr["xtv"][:, t, :]
            p_rz = prz.tile([128, 2 * N], f32, tag="prz")
            p_n = pn.tile([128, N], f32, tag="pn")
            if h is not None:
                p_h = phgn.tile([128, N], f32, tag="phgn")
                mm(p_rz[:, 0:N], bd_g[0], h[:], True, False)
                mm(p_rz[:, 0:N], x2_g[0], xcols, False, True)
                mm(p_rz[:, N:2 * N], bd_g[1], h[:], True, False)
                mm(p_rz[:, N:2 * N], x2_g[1], xcols, False, True)
                mm(p_h[:], bd_g[2], h[:], True, True)
                pr["p_h"] = p_h
            else:
                mm(p_rz[:, 0:N], x2_g[0], xcols, True, True)
                mm(p_rz[:, N:2 * N], x2_g[1], xcols, True, True)
                pr["p_h"] = None
            mm(p_n[:], x2_g[2], xcols, True, False)

            rz = rzp.tile([128, 2 * N], bf16, tag="rz")
            nc.scalar.activation(rz[:, 0:N], p_rz[:, 0:N], Act.Sigmoid, bias=bias_r)
            nc.scalar.activation(rz[:, N:2 * N], p_rz[:, N:2 * N], Act.Sigmoid,
                                 bias=bias_z)
            pr["p_n"], pr["rz"] = p_n, rz

        def stage_b(pr, t):
            """m1 fused mult, identity accumulate, tanh; off-chain w and z*h."""
            p_n, rz, p_h = pr["p_n"], pr["rz"], pr["p_h"]
            m1 = m1p.tile([128, N], bf16, tag="m1")
            if p_h is not None:
                nc.vector.scalar_tensor_tensor(m1[:], p_h[:], b2, rz[:, 0:N],
                                               Alu.add, Alu.mult)
            else:
                nc.vector.tensor_scalar(m1[:], rz[:, 0:N], b2, None, Alu.mult)
            mm(p_n[:], i128, m1[:], False, True)
            w = wp.tile([128, N], bf16, tag="w")
            nc.vector.tensor_scalar(w[:], rz[:, N:2 * N], -1.0, 1.0,
                                    Alu.mult, Alu.add)
            if pr["h"] is not None:
                zh = zhp.tile([128, N], bf16, tag="zh")
                nc.vector.tensor_tensor(zh[:], rz[:, N:2 * N], pr["h"][:], Alu.mult)
                pr["zh"] = zh
            else:
                pr["zh"] = None
            pr["w"] = w
            n_t = np_.tile([128, N], bf16, tag="nt")
            nc.scalar.activation(n_t[:], p_n[:], Act.Tanh, bias=bias_n)
            pr["n_t"] = n_t

        def stage_c(pr, t):
            """h' = n*(1-z) + z*h   (zh precomputed off-chain)."""
            n_t, w, zh = pr["n_t"], pr["w"], pr["zh"]
            h_new = hp.tile([128, N], bf16, tag="h")
            if zh is not None:
                nw = nwp.tile([128, N], bf16, tag="nw")
                nc.vector.tensor_tensor(nw[:], n_t[:], w[:], Alu.mult)
                nc.vector.tensor_tensor(h_new[:], nw[:], zh[:], Alu.add)
            else:
                nc.vector.tensor_tensor(h_new[:], n_t[:], w[:], Alu.mult)
            pr["h"] = h_new

        def fc_out(pr, st, j):
            h = pr["h"]
            for g in range(2):
                p_l = plog.tile([2, N], f32, tag="pn")
                mm(p_l[:], wfc[64 * g:64 * (g + 1), :], h[64 * g:64 * (g + 1), :],
                   True, True)
                stg = st[0:2, j * PAIR + g * N:j * PAIR + (g + 1) * N]
                nc.vector.tensor_scalar(stg, p_l[:], bfc, None, Alu.add)

        for blk in range(NPAIR // IL):
            sbbase = blk * SB
            pairs = []
            st = stp.tile([2, SB], bf16, tag="st")
            for j in range(IL):
                pidx = blk * IL + j
                base = sbbase + j * PAIR
                xt = xt_pool.tile([2, XW], bf16)
                # flat contiguous DMA: 2 descriptors of 15KB
                nc.sync.dma_start(xt[:], xs_d[2 * pidx:2 * pidx + 2, :])
                pairs.append({"xtv": xt[:].rearrange("g (t n) -> g t n", n=N),
                              "base": base, "h": None})
            for t in range(T):
                for pr in pairs:
                    stage_a(pr, t)
                for pr in pairs:
                    stage_b(pr, t)
                for pr in pairs:
                    stage_c(pr, t)
            for j, pr in enumerate(pairs):
                fc_out(pr, st, j)
            nc.sync.dma_start(out_d[0:2, sbbase:sbbase + SB], st[0:2, :])

    nc.compile()
    return nc


def _host_constants(W_ih, W_hh, b_ih, b_hh, W_fc, b_fc):
    import ml_dtypes

    f32 = np.float32
    cbf = np.zeros((128, 1024), f32)
    cf = np.zeros((128, 8), f32)
    w_in = W_ih[:, 0].astype(f32)
    for g in range(3):
        W = W_hh[64 * g:64 * (g + 1)].astype(f32)          # [64, 64]
        cbf[0:64, 128 * g:128 * g + 64] = W.T
        cbf[64:128, 128 * g + 64:128 * g + 128] = W.T
        wg = w_in[64 * g:64 * (g + 1)]
        cbf[0, 514 + 128 * g:514 + 128 * g + 64] = wg
        cbf[1, 514 + 128 * g + 64:514 + 128 * g + 128] = wg
    cbf[:, 384:512] = np.eye(128, dtype=f32)
    cbf[0:64, 512:514] = W_fc.T
    cbf[64:128, 512:514] = W_fc.T
    cf[:, 0] = np.concatenate([(b_ih[0:64] + b_hh[0:64])] * 2)
    cf[:, 1] = np.concatenate([(b_ih[64:128] + b_hh[64:128])] * 2)
    cf[:, 2] = np.concatenate([b_ih[128:192]] * 2)
    cf[:, 3] = np.concatenate([b_hh[128:192]] * 2)
    cf[0:2, 4] = b_fc
    return {"cbf": cbf.astype(ml_dtypes.bfloat16), "cf": cf}


def kernel(x, W_ih, W_hh, b_ih, b_hh, W_fc, b_fc, _trace=False, _trace_kwargs=None):
    from concourse.bass_utils import run_bass_kernel_spmd

    if "nc" not in _cache:
        _cache["nc"] = _build()
    nc = _cache["nc"]

    consts = _host_constants(W_ih, W_hh, b_ih, b_hh, W_fc, b_fc)
    x = np.ascontiguousarray(np.asarray(x, np.float32))
    in_maps = []
    for c in range(NCORES):
        m = {"x": x[c * BC:(c + 1) * BC]}
        m.update(consts)
        in_maps.append(m)
    kw = {}
    if _trace:
        kw["trace"] = True
        if _trace_kwargs:
            kw.update(_trace_kwargs)
    res = run_bass_kernel_spmd(nc, in_maps, list(range(NCORES)), **kw)
    out = np.concatenate(
        [np.asarray(res.results[c]["out"]).astype(np.float32).T
         for c in range(NCORES)], axis=0)
    if _trace:
        return out, res
    return out


if __name__ == "__main__":
    rng = np.random.default_rng(0)
    s = 1.0 / np.sqrt(H)
    inputs = {
        "x": rng.standard_normal((B, T), dtype=np.float32),
        "W_ih": rng.uniform(-s, s, (3 * H, 1)).astype(np.float32),
        "W_hh": rng.uniform(-s, s, (3 * H, H)).astype(np.float32),
        "b_ih": rng.uniform(-s, s, (3 * H,)).astype(np.float32),
        "b_hh": rng.uniform(-s, s, (3 * H,)).astype(np.float32),
        "W_fc": rng.uniform(-s, s, (2, H)).astype(np.float32),
        "b_fc": rng.uniform(-s, s, (2,)).astype(np.float32),
    }
    out = kernel(**inputs)
    print(out.shape, out.dtype, out[:4])


# revision 11
# speedup vs baseline: 3.2930x; 1.1679x over previous
"""Trainium2 Bass kernel for nn_LungCancerGRU (GRU H=64, T=15, B=262144 -> logits [B,2]).

Data parallel over 8 NeuronCores (batch sharded, 32768 rows/core).

Per-core layout: gate units on SBUF partitions, batch on the free dimension.
Batch runs in pair-tiles of 1024 rows = two groups (A, B) of N=512; group A
occupies partitions 0..63, group B 64..127 of every [128, 512] tile.  Two
pair-tiles (IL=2) run in lockstep to hide the recurrence critical path.

All matmuls are bf16 (moving operand dtype determines PE rate; fp32 moving
data costs 4 cycles/column).  x is cast to bf16 once in a wide 128-partition
layout and round-tripped through scratch DRAM so the per-pair transposed
loads are 2 contiguous descriptors instead of 1024 60-byte ones.

Per timestep t (per pair-tile):
  p_rz[:, :512] = BD(W_hr^T) @ h + x2_r @ x_t       (r preact, K=128 + K=2)
  p_rz[:, 512:] = BD(W_hz^T) @ h + x2_z @ x_t       (z preact)
  p_hgn         = BD(W_hn^T) @ h                    (h-part of n gate)
  p_n           = x2_n @ x_t                        (x-part of n gate)
  r   = sigmoid(p_rz[:, :512] + bias_r)             ACT, per-partition bias
  z   = sigmoid(p_rz[:, 512:] + bias_z)
  m1  = (p_hgn + b_hh_n) * r                        DVE scalar_tensor_tensor
  p_n += I128 @ m1                                  identity-matmul accumulate
  n   = tanh(p_n + b_ih_n)                          ACT
  w = 1-z; zh = z*h; nw = n*w; h' = nw + zh         DVE bf16

FC head: logitsT [2, 512] per group via PE (stationary W_fc^T slice), bias
added in the PSUM->SBUF tensor_scalar copy, staged per-superblock and DMA'd
to a transposed [2, BC] bf16 DRAM output; the host transposes back.
"""

import sys

import numpy as np

sys.path.insert(0, "/opt/trn_rl_repo")

B, T, H = 262144, 15, 64
NCORES = 8
BC = B // NCORES          # 32768 rows per core
N = 512                   # batch columns per group
PAIR = 2 * N              # 1024 rows per pair-tile
NPAIR = BC // PAIR        # 32 pair-tiles per core
IL = 2                    # pair-tiles processed in lockstep
XW = T * N                # xt tile free width (7680)
SB = IL * PAIR            # rows per superblock (2048)

_cache = {}


def _build():
    from contextlib import ExitStack

    import concourse.bacc as bacc
    import concourse.mybir as mybir
    from concourse.tile import TileContext

    f32 = mybir.dt.float32
    bf16 = mybir.dt.bfloat16
    Act = mybir.ActivationFunctionType
    Alu = mybir.AluOpType

    nc = bacc.Bacc(None)

    x_in = nc.dram_tensor("x", [BC, T], f32, kind="ExternalInput")
    out_d = nc.dram_tensor("out", [2, BC], bf16, kind="ExternalOutput")
    cbf_in = nc.dram_tensor("cbf", [128, 1024], bf16, kind="ExternalInput")
    cf_in = nc.dram_tensor("cf", [128, 8], f32, kind="ExternalInput")
    # scratch x, bf16, t-major per 512-row group-block: xs[r, t*512+n] = x[512r+n, t]
    xs_d = nc.dram_tensor("xs", [BC // N, XW], bf16, kind="Internal")

    with TileContext(nc) as tc, ExitStack() as es:
        # ---- constants ----
        cpool = es.enter_context(tc.tile_pool(name="const", bufs=1))
        cbf = cpool.tile([128, 1024], bf16)
        nc.sync.dma_start(cbf[:], cbf_in[:])
        cf = cpool.tile([128, 8], f32)
        nc.sync.dma_start(cf[:], cf_in[:])

        bd_g = [cbf[:, 128 * g:128 * (g + 1)] for g in range(3)]
        i128 = cbf[:, 384:512]
        wfc = cbf[:, 512:514]
        x2_g = [cbf[0:2, 514 + 128 * g:514 + 128 * (g + 1)] for g in range(3)]
        bias_r = cf[:, 0:1]
        bias_z = cf[:, 1:2]
        bias_n = cf[:, 2:3]
        b2 = cf[:, 3:4]
        bfc = cf[0:2, 4:5]

        # ---- x pre-pass: f32 wide load -> transposing bf16 cast -> scratch ----
        xpre = es.enter_context(tc.tile_pool(name="xpre", bufs=1))
        xw = xpre.tile([64, XW], f32)
        nc.sync.dma_start(xw[:], x_in[:].rearrange("(p n) t -> p (n t)", p=64))
        xbw = xpre.tile([64, XW], bf16)
        nc.vector.tensor_copy(xbw[:].rearrange("p (t n) -> p t n", n=N),
                              xw[:].rearrange("p (n t) -> p t n", t=T))
        nc.sync.dma_start(xs_d[:], xbw[:])

        # ---- pools ----
        xt_pool = es.enter_context(tc.tile_pool(name="xt", bufs=3))
        hp = es.enter_context(tc.tile_pool(name="h", bufs=2 * IL))
        rzp = es.enter_context(tc.tile_pool(name="rz", bufs=2 * IL))
        m1p = es.enter_context(tc.tile_pool(name="m1", bufs=2 * IL))
        np_ = es.enter_context(tc.tile_pool(name="nt", bufs=2 * IL))
        wp = es.enter_context(tc.tile_pool(name="w", bufs=2 * IL))
        zhp = es.enter_context(tc.tile_pool(name="zh", bufs=2 * IL))
        nwp = es.enter_context(tc.tile_pool(name="nw", bufs=2 * IL))
        stp = es.enter_context(tc.tile_pool(name="stage", bufs=2))
        prz = es.enter_context(tc.tile_pool(name="prz", bufs=2, space="PSUM"))
        pn = es.enter_context(tc.tile_pool(name="pn", bufs=3, space="PSUM"))
        phgn = es.enter_context(tc.tile_pool(name="phgn", bufs=1, space="PSUM"))
        plog = pn  # FC logits rotate through the pn slots (shared tag)

        def mm(out, lhsT, rhs, start, stop):
            nc.tensor.matmul(out, lhsT, rhs, start=start, stop=stop,
                             skip_group_check=True)

        # ---- engine warm-ups: fold const-DMA sems into each engine's clock
        pwarm = plog.tile([2, 2], f32, tag="pn")
        mm(pwarm[:], cbf[0:2, 0:2], cbf[0:2, 0:2], True, True)
        wt = cpool.tile([2, 8], f32)
        nc.vector.tensor_copy(wt[0:1, 0:1], cf[0:1, 0:1])
        nc.vector.tensor_copy(wt[0:1, 1:2], cbf[0:1, 0:1])
        nc.scalar.copy(wt[0:1, 2:3], cf[0:1, 0:1])
        nc.scalar.copy(wt[0:1, 3:4], cbf[0:1, 0:1])

        def stage_x(pr, t, first):
            """x-contribution matmuls for step t — no h dependency, so these
            fill the PE bubble while the previous step's h' chain finishes."""
            xcols = pr["xtv"][:, t, :]
            p_rz = prz.tile([128, 2 * N], f32, tag="prz")
            p_n = pn.tile([128, N], f32, tag="pn")
            mm(p_rz[:, 0:N], x2_g[0], xcols, True, first)
            mm(p_rz[:, N:2 * N], x2_g[1], xcols, True, first)
            mm(p_n[:], x2_g[2], xcols, True, False)
            pr["p_rz"], pr["p_n"] = p_rz, p_n

        def stage_a(pr, t):
            """h matmuls (accumulate onto x preacts) + sigmoids."""
            h = pr["h"]
            p_rz = pr["p_rz"]
            if h is not None:
                p_h = phgn.tile([128, N], f32, tag="phgn")
                mm(p_rz[:, 0:N], bd_g[0], h[:], False, True)
                mm(p_rz[:, N:2 * N], bd_g[1], h[:], False, True)
                mm(p_h[:], bd_g[2], h[:], True, True)
                pr["p_h"] = p_h
            else:
                pr["p_h"] = None

            rz = rzp.tile([128, 2 * N], bf16, tag="rz")
            nc.scalar.activation(rz[:, 0:N], p_rz[:, 0:N], Act.Sigmoid, bias=bias_r)
            nc.scalar.activation(rz[:, N:2 * N], p_rz[:, N:2 * N], Act.Sigmoid,
                                 bias=bias_z)
            pr["rz"] = rz

        def stage_b(pr, t):
            """m1 fused mult, identity accumulate, tanh; off-chain w and z*h."""
            p_n, rz, p_h = pr["p_n"], pr["rz"], pr["p_h"]
            m1 = m1p.tile([128, N], bf16, tag="m1")
            if p_h is not None:
                nc.vector.scalar_tensor_tensor(m1[:], p_h[:], b2, rz[:, 0:N],
                                               Alu.add, Alu.mult)
            else:
                nc.vector.tensor_scalar(m1[:], rz[:, 0:N], b2, None, Alu.mult)
            mm(p_n[:], i128, m1[:], False, True)
            w = wp.tile([128, N], bf16, tag="w")
            nc.vector.tensor_scalar(w[:], rz[:, N:2 * N], -1.0, 1.0,
                                    Alu.mult, Alu.add)
            if pr["h"] is not None:
                zh = zhp.tile([128, N], bf16, tag="zh")
                nc.gpsimd.tensor_tensor(zh[:], rz[:, N:2 * N], pr["h"][:], Alu.mult)
                pr["zh"] = zh
            else:
                pr["zh"] = None
            pr["w"] = w
            n_t = np_.tile([128, N], bf16, tag="nt")
            nc.scalar.activation(n_t[:], p_n[:], Act.Tanh, bias=bias_n)
            pr["n_t"] = n_t

        def stage_c(pr, t):
            """h' = n*(1-z) + z*h   (zh precomputed off-chain)."""
            n_t, w, zh = pr["n_t"], pr["w"], pr["zh"]
            h_new = hp.tile([128, N], bf16, tag="h")
            if zh is not None:
                nw = nwp.tile([128, N], bf16, tag="nw")
                nc.vector.tensor_tensor(nw[:], n_t[:], w[:], Alu.mult)
                nc.vector.tensor_tensor(h_new[:], nw[:], zh[:], Alu.add)
            else:
                nc.vector.tensor_tensor(h_new[:], n_t[:], w[:], Alu.mult)
            pr["h"] = h_new

        def fc_out(pr, st, j):
            h = pr["h"]
            for g in range(2):
                p_l = plog.tile([2, N], f32, tag="pn")
                mm(p_l[:], wfc[64 * g:64 * (g + 1), :], h[64 * g:64 * (g + 1), :],
                   True, True)
                stg = st[0:2, j * PAIR + g * N:j * PAIR + (g + 1) * N]
                nc.vector.tensor_scalar(stg, p_l[:], bfc, None, Alu.add)

        for blk in range(NPAIR // IL):
            sbbase = blk * SB
            pairs = []
            st = stp.tile([2, SB], bf16, tag="st")
            for j in range(IL):
                pidx = blk * IL + j
                base = sbbase + j * PAIR
                xt = xt_pool.tile([2, XW], bf16)
                # flat contiguous DMA: 2 descriptors of 15KB
                nc.sync.dma_start(xt[:], xs_d[2 * pidx:2 * pidx + 2, :])
                pairs.append({"xtv": xt[:].rearrange("g (t n) -> g t n", n=N),
                              "base": base, "h": None})
            for pr in pairs:
                stage_x(pr, 0, True)
            for t in range(T):
                for pr in pairs:
                    stage_a(pr, t)
                for pr in pairs:
                    stage_b(pr, t)
                if t < T - 1:
                    for pr in pairs:
                        stage_x(pr, t + 1, False)
                for pr in pairs:
                    stage_c(pr, t)
            for j, pr in enumerate(pairs):
                fc_out(pr, st, j)
            nc.sync.dma_start(out_d[0:2, sbbase:sbbase + SB], st[0:2, :])

    nc.compile()
    return nc


def _host_constants(W_ih, W_hh, b_ih, b_hh, W_fc, b_fc):
    import ml_dtypes

    f32 = np.float32
    cbf = np.zeros((128, 1024), f32)
    cf = np.zeros((128, 8), f32)
    w_in = W_ih[:, 0].astype(f32)
    for g in range(3):
        W = W_hh[64 * g:64 * (g + 1)].astype(f32)          # [64, 64]
        cbf[0:64, 128 * g:128 * g + 64] = W.T
        cbf[64:128, 128 * g + 64:128 * g + 128] = W.T
        wg = w_in[64 * g:64 * (g + 1)]
        cbf[0, 514 + 128 * g:514 + 128 * g + 64] = wg
        cbf[1, 514 + 128 * g + 64:514 + 128 * g + 128] = wg
    cbf[:, 384:512] = np.eye(128, dtype=f32)
    cbf[0:64, 512:514] = W_fc.T
    cbf[64:128, 512:514] = W_fc.T
    cf[:, 0] = np.concatenate([(b_ih[0:64] + b_hh[0:64])] * 2)
    cf[:, 1] = np.concatenate([(b_ih[64:128] + b_hh[64:128])] * 2)
    cf[:, 2] = np.concatenate([b_ih[128:192]] * 2)
    cf[:, 3] = np.concatenate([b_hh[128:192]] * 2)
    cf[0:2, 4] = b_fc
    return {"cbf": cbf.astype(ml_dtypes.bfloat16), "cf": cf}


def kernel(x, W_ih, W_hh, b_ih, b_hh, W_fc, b_fc, _trace=False, _trace_kwargs=None):
    from concourse.bass_utils import run_bass_kernel_spmd

    if "nc" not in _cache:
        _cache["nc"] = _build()
    nc = _cache["nc"]

    consts = _host_constants(W_ih, W_hh, b_ih, b_hh, W_fc, b_fc)
    x = np.ascontiguousarray(np.asarray(x, np.float32))
    in_maps = []
    for c in range(NCORES):
        m = {"x": x[c * BC:(c + 1) * BC]}
        m.update(consts)
        in_maps.append(m)
    kw = {}
    if _trace:
        kw["trace"] = True
        if _trace_kwargs:
            kw.update(_trace_kwargs)
    res = run_bass_kernel_spmd(nc, in_maps, list(range(NCORES)), **kw)
    out = np.concatenate(
        [np.asarray(res.results[c]["out"]).astype(np.float32).T
         for c in range(NCORES)], axis=0)
    if _trace:
        return out, res
    return out


if __name__ == "__main__":
    rng = np.random.default_rng(0)
    s = 1.0 / np.sqrt(H)
    inputs = {
        "x": rng.standard_normal((B, T), dtype=np.float32),
        "W_ih": rng.uniform(-s, s, (3 * H, 1)).astype(np.float32),
        "W_hh": rng.uniform(-s, s, (3 * H, H)).astype(np.float32),
        "b_ih": rng.uniform(-s, s, (3 * H,)).astype(np.float32),
        "b_hh": rng.uniform(-s, s, (3 * H,)).astype(np.float32),
        "W_fc": rng.uniform(-s, s, (2, H)).astype(np.float32),
        "b_fc": rng.uniform(-s, s, (2,)).astype(np.float32),
    }
    out = kernel(**inputs)
    print(out.shape, out.dtype, out[:4])
